# revision 21
# baseline (speedup 1.0000x reference)
"""Trainium2 Bass kernel for attention with per-head qk-layernorm.

Problem (hardcoded): B=2, N=4096, C=1024, H=16, D=64, f32 I/O.
  qkv = x @ qkv_w.T + qkv_b ; per-head LN(q), LN(k) (eps 1e-5)
  attn = softmax(q*D^-0.5 @ k.T) @ v ; out = attn @ proj_w.T + proj_b

Sharding (8 cores): core c -> batch b=c//4, query rows [1024*(c%4), +1024).
Each core computes q,k,v for its own 1024 rows (all 16 heads), AllGathers
k^T/v across its 4-core batch group, runs flash attention for its query rows
over the full 4096-key sequence, and projects. Output needs no collective:
host concatenates the 8 [1024,1024] slices.

Numerics: matmuls bf16 with f32 PSUM accumulation. Softmax skips
max-subtraction: LN guarantees ||q_row||,||k_row|| <= sqrt(D)=8, so
|S| = |q.k|*D^-0.5 <= 8 -> exp safe in f32. Softmax denominators come from a
ones-column appended to V (row 64 of the PV accumulator).
"""

import os
import sys

for _p in ("/opt/trn_rl_repo", "/root/.axon_site/_ro/trn_rl_repo"):
    if os.path.isdir(_p) and _p not in sys.path:
        sys.path.insert(0, _p)

import numpy as np
import ml_dtypes

B, N, C = 2, 4096, 1024
H, D = 16, 64
NLOC = N // 4          # query rows per core = 1024
P = 128                # partitions
LN_EPS = 1e-5
SCALE = D ** -0.5
N_CORES = 8
BF16 = ml_dtypes.bfloat16

_COMPILED = {}


def build_graph():
    import concourse.bass as bass
    import concourse.mybir as mybir
    import concourse.tile as tile
    from concourse import bacc
    from concourse.masks import make_identity

    fp32 = mybir.dt.float32
    bf16 = mybir.dt.bfloat16
    AF = mybir.ActivationFunctionType
    ALU = mybir.AluOpType
    AX = mybir.AxisListType

    nc = bacc.Bacc(trn_type="TRN2", target_bir_lowering=False, num_devices=N_CORES)

    # ---- I/O -------------------------------------------------------------
    xT = nc.declare_dram_parameter("xT", [C, NLOC], bf16, isOutput=False)          # x slice, transposed
    wqkvT = nc.declare_dram_parameter("wqkvT", [C, 3 * C], bf16, isOutput=False)   # qkv_w.T
    qkvb = nc.declare_dram_parameter("qkvb", [1, 3 * C], fp32, isOutput=False)
    wpT = nc.declare_dram_parameter("wpT", [C, C], bf16, isOutput=False)           # proj_w.T
    pb = nc.declare_dram_parameter("pb", [1, C], fp32, isOutput=False)
    qn_wb = nc.declare_dram_parameter("qn_wb", [D, 2], fp32, isOutput=False)       # [:,0]=w [:,1]=b
    kn_wb = nc.declare_dram_parameter("kn_wb", [D, 2], fp32, isOutput=False)
    out = nc.declare_dram_parameter("out", [NLOC, C], fp32, isOutput=True)

    NT = NLOC // P        # 8 local row tiles
    HP = H // 2           # 8 head pairs
    KT = N // P           # 32 key tiles
    CH3 = 3 * C // 512    # 6 qkv channel chunks of 512

    rg = [[0, 1, 2, 3], [4, 5, 6, 7]]

    with tile.TileContext(nc) as tc:
        # ---------- persistent pools ----------
        with (
            tc.tile_pool(name="const", bufs=1) as const,
            tc.tile_pool(name="persist", bufs=1) as persist,
            tc.tile_pool(name="dram", bufs=1, space="DRAM") as dram,
        ):
            ident = const.tile([P, P], bf16, tag="ident", name="ident")
            make_identity(nc, ident)
            ones_row = const.tile([1, P], bf16, tag="ones_row", name="ones_row")
            nc.any.memset(ones_row[:], 1.0)
            eps_t = const.tile([P, 1], fp32, tag="eps_t", name="eps_t")
            nc.any.memset(eps_t[:], LN_EPS)

            qkvb_f = const.tile([1, 3 * C], fp32, tag="qkvb_f", name="qkvb_f")
            nc.sync.dma_start(qkvb_f[:], qkvb[:])
            qkvb_bf = const.tile([1, 3 * C], bf16, tag="qkvb_bf", name="qkvb_bf")
            nc.vector.tensor_copy(qkvb_bf[:], qkvb_f[:])
            pb_f = const.tile([1, C], fp32, tag="pb_f", name="pb_f")
            nc.sync.dma_start(pb_f[:], pb[:])
            pb_bf = const.tile([1, C], bf16, tag="pb_bf", name="pb_bf")
            nc.vector.tensor_copy(pb_bf[:], pb_f[:])
            qnwb = const.tile([D, 2], fp32, tag="qnwb", name="qnwb")
            nc.sync.dma_start(qnwb[:], qn_wb[:])
            knwb = const.tile([D, 2], fp32, tag="knwb", name="knwb")
            nc.sync.dma_start(knwb[:], kn_wb[:])

            # qT / kT-local / attnT accumulators (head-pair-major layout)
            qT_sb = [persist.tile([P, NLOC], bf16, tag=f"qT{p}", name=f"qT{p}") for p in range(HP)]
            attnT = [persist.tile([P, NLOC], bf16, tag=f"aT{p}", name=f"aT{p}") for p in range(HP)]

            kv_local = dram.tile([2 * NLOC, C], bf16, tag="kv_local", name="kv_local")
            kv_full = dram.tile([2 * N, C], bf16, tag="kv_full", name="kv_full")

            # ================= Phase A: QKV + LN + transposes =================
            with (
                tc.tile_pool(name="qkv_ps", bufs=3, space="PSUM") as qkv_ps,
                tc.tile_pool(name="tp_ps", bufs=2, space="PSUM") as tp_ps,
                tc.tile_pool(name="ln", bufs=2) as ln_pool,
                tc.tile_pool(name="kv_stage", bufs=2) as kv_stage,
                tc.tile_pool(name="pa_w", bufs=1) as pa_w,
            ):
                xT_sb = [pa_w.tile([P, NLOC], bf16, tag=f"xT{i}", name=f"xT{i}") for i in range(8)]
                for i in range(8):
                    nc.sync.dma_start(xT_sb[i][:], xT[i * P:(i + 1) * P, :])
                wq_sb = [pa_w.tile([P, 3 * C], bf16, tag=f"wq{i}", name=f"wq{i}") for i in range(8)]
                for i in range(8):
                    nc.sync.dma_start(wq_sb[i][:], wqkvT[i * P:(i + 1) * P, :])
                for i in range(NT):
                    q_f = ln_pool.tile([P, C], fp32, tag="q_f", name="q_f")
                    k_f = ln_pool.tile([P, C], fp32, tag="k_f", name="k_f")
                    v_bf = kv_stage.tile([P, C], bf16, tag="v_bf", name="v_bf")
                    for j in range(CH3):
                        ps = qkv_ps.tile([P, 512], fp32, tag="ps", name="ps")
                        nc.tensor.matmul(ps[:], ones_row[:, :P],
                                         qkvb_bf[:, j * 512:(j + 1) * 512],
                                         start=True, stop=False)
                        for kk in range(8):
                            nc.tensor.matmul(
                                ps[:],
                                xT_sb[kk][:, i * P:(i + 1) * P],
                                wq_sb[kk][:, j * 512:(j + 1) * 512],
                                start=False, stop=(kk == 7))
                        if j < 2:
                            nc.vector.tensor_copy(q_f[:, j * 512:(j + 1) * 512], ps[:])
                        elif j < 4:
                            nc.vector.tensor_copy(k_f[:, (j - 2) * 512:(j - 1) * 512], ps[:])
                        else:
                            nc.vector.tensor_copy(v_bf[:, (j - 4) * 512:(j - 3) * 512], ps[:])
                    # v straight out to kv_local rows [NLOC + i*128)
                    nc.sync.dma_start(kv_local[NLOC + i * P: NLOC + (i + 1) * P, :], v_bf[:])

                    for name, t_f, wb in (("q", q_f, qnwb), ("k", k_f, knwb)):
                        t3 = t_f[:].rearrange("p (h d) -> p h d", d=D)
                        sums = ln_pool.tile([P, H], fp32, tag=f"{name}sum", name=f"{name}sum")
                        nc.vector.tensor_reduce(sums[:], t3, axis=AX.X, op=ALU.add)
                        sq = ln_pool.tile([P, C], fp32, tag=f"{name}sq", name=f"{name}sq")
                        nc.scalar.activation(sq[:], t_f[:], AF.Square)
                        ssq = ln_pool.tile([P, H], fp32, tag=f"{name}ssq", name=f"{name}ssq")
                        nc.vector.tensor_reduce(
                            ssq[:], sq[:].rearrange("p (h d) -> p h d", d=D),
                            axis=AX.X, op=ALU.add)
                        mu = ln_pool.tile([P, H], fp32, tag=f"{name}mu", name=f"{name}mu")
                        nc.vector.tensor_scalar_mul(mu[:], sums[:], 1.0 / D)
                        mu2 = ln_pool.tile([P, H], fp32, tag=f"{name}mu2", name=f"{name}mu2")
                        nc.vector.tensor_mul(mu2[:], mu[:], mu[:])
                        var = ln_pool.tile([P, H], fp32, tag=f"{name}var", name=f"{name}var")
                        nc.vector.scalar_tensor_tensor(
                            var[:], ssq[:], 1.0 / D, mu2[:],
                            op0=ALU.mult, op1=ALU.subtract)
                        sig = ln_pool.tile([P, H], fp32, tag=f"{name}sig", name=f"{name}sig")
                        nc.scalar.activation(sig[:], var[:], AF.Sqrt, bias=eps_t[:])
                        rstd = ln_pool.tile([P, H], fp32, tag=f"{name}rstd", name=f"{name}rstd")
                        nc.vector.reciprocal(rstd[:], sig[:])
                        tn = ln_pool.tile([P, C], bf16, tag=f"{name}n", name=f"{name}n")
                        for h in range(H):
                            nc.vector.tensor_scalar(
                                tn[:, h * D:(h + 1) * D], t_f[:, h * D:(h + 1) * D],
                                mu[:, h:h + 1], rstd[:, h:h + 1],
                                op0=ALU.subtract, op1=ALU.mult)
                        # transpose per head into [d, n] layout (+ LN affine)
                        for h in range(H):
                            tp = tp_ps.tile([D, P], bf16, tag="tp", name="tp")
                            nc.tensor.transpose(tp[:], tn[:, h * D:(h + 1) * D], ident[:])
                            hp, hh = h // 2, h % 2
                            if name == "q":
                                nc.vector.tensor_scalar(
                                    qT_sb[hp][hh * D:(hh + 1) * D, i * P:(i + 1) * P],
                                    tp[:], wb[:, 0:1], wb[:, 1:2],
                                    op0=ALU.mult, op1=ALU.add)
                            else:
                                if h == 0:
                                    kT_stage = kv_stage.tile([P, C], bf16,
                                                             tag="kT_stage", name="kT_stage")
                                nc.vector.tensor_scalar(
                                    kT_stage[hh * D:(hh + 1) * D, hp * P:(hp + 1) * P],
                                    tp[:], wb[:, 0:1], wb[:, 1:2],
                                    op0=ALU.mult, op1=ALU.add)
                        if name == "k":
                            for hp in range(HP):
                                nc.sync.dma_start(
                                    kv_local[hp * P:(hp + 1) * P, i * P:(i + 1) * P],
                                    kT_stage[:, hp * P:(hp + 1) * P])

            # ================= Phase B: AllGather k^T | v =====================
            nc.gpsimd.collective_compute(
                "AllGather", mybir.AluOpType.bypass,
                replica_groups=rg,
                ins=[kv_local[:].opt()],
                outs=[kv_full[:].opt()],
            )

            # ================= Phase C: flash attention =======================
            SL = 2 * (D + 1)   # 130: [vA(64)|1|vB(64)|1] per key tile
            with (
                tc.tile_pool(name="st_ps", bufs=2, space="PSUM") as st_ps,
                tc.tile_pool(name="o_ps", bufs=2, space="PSUM") as o_ps,
                tc.tile_pool(name="kv_sb", bufs=2) as kv_sb,
                tc.tile_pool(name="p_sb", bufs=2) as p_sb,
                tc.tile_pool(name="nrm", bufs=2) as nrm,
            ):
                for hp in range(HP):
                    kThp = kv_sb.tile([P, N], bf16, tag="kThp", name="kThp")
                    for r in range(4):
                        nc.sync.dma_start(
                            kThp[:, r * NLOC:(r + 1) * NLOC],
                            kv_full[2 * NLOC * r + hp * P: 2 * NLOC * r + (hp + 1) * P, :])
                    vaug = kv_sb.tile([P, KT * SL], bf16, tag="vaug", name="vaug")
                    # ones-columns at 64, 129, 194, ... (step 65 covers both halves)
                    nc.vector.memset(vaug[:, D::(D + 1)], 1.0)
                    for r in range(4):
                        for hh in range(2):
                            nc.sync.dma_start(
                                vaug[:, r * 8 * SL:(r + 1) * 8 * SL].rearrange(
                                    "p (t d) -> p t d", d=SL)[
                                    :, :, hh * (D + 1): hh * (D + 1) + D],
                                kv_full[2 * NLOC * r + NLOC: 2 * NLOC * r + 2 * NLOC,
                                        hp * P + hh * D: hp * P + (hh + 1) * D].rearrange(
                                    "(t p) d -> p t d", p=P))
                    for m in range(2):
                        o_tiles = [o_ps.tile([D + 1, 512], fp32, tag=f"o{hh}", name=f"o{hh}")
                                   for hh in range(2)]
                        # software pipeline: S+exp run LEAD steps ahead of PV so
                        # PE never stalls in-order behind a pending exp.
                        LEAD = 2
                        pq = []  # pending (t2, hh, p_tile)

                        def issue_pv(t4p, hh, p_t):
                            for u in range(4):
                                t = 4 * t4p + u
                                nc.tensor.matmul(
                                    o_tiles[hh][:],
                                    vaug[:, t * SL + hh * (D + 1):
                                         t * SL + (hh + 1) * (D + 1)],
                                    p_t[:, u * 512:(u + 1) * 512],
                                    start=(t == 0), stop=(t == KT - 1))

                        # S tiles: PE -> PSUM -> (DVE) -> wide SBUF f32 staging ->
                        # one big ACT exp per 4 key-tiles -> PV.  DVE ferry keeps
                        # ACT ops wide (amortizes the ~352-cycle ACT overhead).
                        for t4 in range(KT // 4):
                            for hh in range(2):
                                s_stage = p_sb.tile([P, 2048], fp32, tag="s_stage",
                                                    name="s_stage")
                                for v2 in range(2):
                                    st = st_ps.tile([P, 1024], fp32, tag="st", name="st")
                                    for u in range(2):
                                        t = 4 * t4 + 2 * v2 + u
                                        nc.tensor.matmul(
                                            st[:, u * 512:(u + 1) * 512],
                                            kThp[hh * D:(hh + 1) * D, t * P:(t + 1) * P],
                                            qT_sb[hp][hh * D:(hh + 1) * D,
                                                      m * 512:(m + 1) * 512],
                                            start=True, stop=True)
                                    nc.vector.tensor_copy(
                                        s_stage[:, v2 * 1024:(v2 + 1) * 1024], st[:])
                                p_t = p_sb.tile([P, 2048], bf16, tag=f"p{hh}",
                                                name=f"p{hh}")
                                nc.scalar.activation(p_t[:], s_stage[:], AF.Exp,
                                                     scale=SCALE)
                                pq.append((t4, hh, p_t))
                            while len(pq) > LEAD:
                                issue_pv(*pq.pop(0))
                        for args in pq:
                            issue_pv(*args)
                        for hh in range(2):
                            linv = nrm.tile([1, 512], fp32, tag=f"li{hh}", name=f"li{hh}")
                            nc.vector.reciprocal(linv[:], o_tiles[hh][D:D + 1, :])
                            bc_sb = nrm.tile([D, 512], fp32, tag=f"bs{hh}", name=f"bs{hh}")
                            nc.gpsimd.partition_broadcast(bc_sb[:], linv[:], channels=D)
                            nc.vector.tensor_mul(
                                attnT[hp][hh * D:(hh + 1) * D, m * 512:(m + 1) * 512],
                                o_tiles[hh][0:D, :], bc_sb[:])

            # ================= Phase D: output projection =====================
            with (
                tc.tile_pool(name="y_ps", bufs=2, space="PSUM") as y_ps,
                tc.tile_pool(name="y_sb", bufs=2) as y_sb_pool,
                tc.tile_pool(name="pd_w", bufs=1) as pd_w,
            ):
                wp_sb = [pd_w.tile([P, C], bf16, tag=f"wp{i}", name=f"wp{i}") for i in range(8)]
                for i in range(8):
                    nc.sync.dma_start(wp_sb[i][:], wpT[i * P:(i + 1) * P, :])
                for i in range(NT):
                    y_sb = y_sb_pool.tile([P, C], fp32, tag="y", name="y")
                    for co in range(2):
                        yp = y_ps.tile([P, 512], fp32, tag="yp", name="yp")
                        nc.tensor.matmul(yp[:], ones_row[:, :P],
                                         pb_bf[:, co * 512:(co + 1) * 512],
                                         start=True, stop=False)
                        for p in range(8):
                            nc.tensor.matmul(
                                yp[:],
                                attnT[p][:, i * P:(i + 1) * P],
                                wp_sb[p][:, co * 512:(co + 1) * 512],
                                start=False, stop=(p == 7))
                        nc.vector.tensor_copy(y_sb[:, co * 512:(co + 1) * 512], yp[:])
                    nc.sync.dma_start(out[i * P:(i + 1) * P, :], y_sb[:])

    nc.finalize()
    return nc


def _prep_in_maps(x, qkv_w, qkv_b, q_norm_w, q_norm_b, k_norm_w, k_norm_b,
                  proj_w, proj_b):
    wqkvT = np.ascontiguousarray(qkv_w.T).astype(BF16)
    wpT = np.ascontiguousarray(proj_w.T).astype(BF16)
    qkvb = qkv_b.reshape(1, 3 * C).astype(np.float32)
    pb = proj_b.reshape(1, C).astype(np.float32)
    qn_wb = np.stack([q_norm_w, q_norm_b], axis=1).astype(np.float32)
    kn_wb = np.stack([k_norm_w, k_norm_b], axis=1).astype(np.float32)
    in_maps = []
    for c in range(N_CORES):
        b, s = c // 4, c % 4
        xt = np.ascontiguousarray(x[b, s * NLOC:(s + 1) * NLOC, :].T).astype(BF16)
        in_maps.append({
            "xT": xt, "wqkvT": wqkvT, "qkvb": qkvb, "wpT": wpT, "pb": pb,
            "qn_wb": qn_wb, "kn_wb": kn_wb,
        })
    return in_maps


def _install_ntff_hook_shim():
    """The agent image's antenv lacks axon_hooks; recreate it so trace=True
    can register the NTFF profile hook that trn_boot would have set."""
    import types
    import antenv

    if "antenv.axon_hooks" in sys.modules:
        return
    mod = types.ModuleType("antenv.axon_hooks")
    state = {"fn": None}
    mod.set_axon_ntff_profile_hook = lambda fn: state.__setitem__("fn", fn)
    mod.get_axon_ntff_profile_hook = lambda: state["fn"]
    sys.modules["antenv.axon_hooks"] = mod
    antenv.axon_hooks = mod
    try:
        from trn_agent_boot.trn_boot import _ntff_profile_via_ctypes
        hook = _ntff_profile_via_ctypes("/opt/axon/libaxon_pjrt.so")
        if hook is not None:
            mod.set_axon_ntff_profile_hook(hook)
    except Exception as e:  # degrade to no tracing
        print(f"ntff hook shim failed: {e}", file=sys.stderr)


def kernel(x, qkv_w, qkv_b, q_norm_w, q_norm_b, k_norm_w, k_norm_b,
           proj_w, proj_b, _trace=False):
    from concourse.bass_utils import run_bass_kernel_spmd

    if _trace:
        _install_ntff_hook_shim()

    if "nc" not in _COMPILED:
        _COMPILED["nc"] = build_graph()
    nc = _COMPILED["nc"]

    in_maps = _prep_in_maps(x, qkv_w, qkv_b, q_norm_w, q_norm_b,
                            k_norm_w, k_norm_b, proj_w, proj_b)
    res = run_bass_kernel_spmd(nc, in_maps, core_ids=list(range(N_CORES)),
                               trace=_trace)
    out = np.empty((B, N, C), dtype=np.float32)
    for c in range(N_CORES):
        b, s = c // 4, c % 4
        out[b, s * NLOC:(s + 1) * NLOC, :] = res.results[c]["out"]
    if _trace:
        _COMPILED["last_exec_time_ns"] = res.exec_time_ns
        _COMPILED["last_results"] = res
    return out


# revision 24
# speedup vs baseline: 1.3553x; 1.3553x over previous
"""Trainium2 Bass kernel for attention with per-head qk-layernorm.

Problem (hardcoded): B=2, N=4096, C=1024, H=16, D=64, f32 I/O.
  qkv = x @ qkv_w.T + qkv_b ; per-head LN(q), LN(k) (eps 1e-5)
  attn = softmax(q*D^-0.5 @ k.T) @ v ; out = attn @ proj_w.T + proj_b

Sharding (8 cores): core c -> batch b=c//4, query rows [1024*(c%4), +1024).
Each core computes q,k,v for its own 1024 rows (all 16 heads), AllGathers
k^T/v across its 4-core batch group, runs flash attention for its query rows
over the full 4096-key sequence, and projects. Output needs no collective:
host concatenates the 8 [1024,1024] slices.

Numerics: matmuls bf16 with f32 PSUM accumulation. Softmax skips
max-subtraction: LN guarantees ||q_row||,||k_row|| <= sqrt(D)=8, so
|S| = |q.k|*D^-0.5 <= 8 -> exp safe in f32. Softmax denominators come from a
ones-column appended to V (row 64 of the PV accumulator).
"""

import os
import sys

for _p in ("/opt/trn_rl_repo", "/root/.axon_site/_ro/trn_rl_repo"):
    if os.path.isdir(_p) and _p not in sys.path:
        sys.path.insert(0, _p)

import numpy as np
import ml_dtypes

B, N, C = 2, 4096, 1024
H, D = 16, 64
NLOC = N // 4          # query rows per core = 1024
P = 128                # partitions
LN_EPS = 1e-5
SCALE = D ** -0.5
N_CORES = 8
BF16 = ml_dtypes.bfloat16

_COMPILED = {}


def build_graph():
    import concourse.bass as bass
    import concourse.mybir as mybir
    import concourse.tile as tile
    from concourse import bacc
    from concourse.masks import make_identity

    fp32 = mybir.dt.float32
    bf16 = mybir.dt.bfloat16
    AF = mybir.ActivationFunctionType
    ALU = mybir.AluOpType
    AX = mybir.AxisListType

    nc = bacc.Bacc(trn_type="TRN2", target_bir_lowering=False, num_devices=N_CORES)

    # ---- I/O -------------------------------------------------------------
    xT = nc.declare_dram_parameter("xT", [C, NLOC], bf16, isOutput=False)          # x slice, transposed
    wqkvT = nc.declare_dram_parameter("wqkvT", [C, 3 * C], bf16, isOutput=False)   # qkv_w.T
    qkvb = nc.declare_dram_parameter("qkvb", [1, 3 * C], fp32, isOutput=False)
    wpT = nc.declare_dram_parameter("wpT", [C, C], bf16, isOutput=False)           # proj_w.T
    pb = nc.declare_dram_parameter("pb", [1, C], fp32, isOutput=False)
    qn_wb = nc.declare_dram_parameter("qn_wb", [D, 2], fp32, isOutput=False)       # [:,0]=w [:,1]=b
    kn_wb = nc.declare_dram_parameter("kn_wb", [D, 2], fp32, isOutput=False)
    out = nc.declare_dram_parameter("out", [NLOC, C], fp32, isOutput=True)

    NT = NLOC // P        # 8 local row tiles
    HP = H // 2           # 8 head pairs
    KT = N // P           # 32 key tiles
    CH3 = 3 * C // 512    # 6 qkv channel chunks of 512

    rg = [[0, 1, 2, 3], [4, 5, 6, 7]]

    with tile.TileContext(nc) as tc:
        # ---------- persistent pools ----------
        with (
            tc.tile_pool(name="const", bufs=1) as const,
            tc.tile_pool(name="persist", bufs=1) as persist,
            tc.tile_pool(name="dram", bufs=1, space="DRAM") as dram,
        ):
            ident = const.tile([P, P], bf16, tag="ident", name="ident")
            make_identity(nc, ident)
            ones_row = const.tile([1, P], bf16, tag="ones_row", name="ones_row")
            nc.any.memset(ones_row[:], 1.0)
            eps_t = const.tile([P, 1], fp32, tag="eps_t", name="eps_t")
            nc.any.memset(eps_t[:], LN_EPS)

            qkvb_f = const.tile([1, 3 * C], fp32, tag="qkvb_f", name="qkvb_f")
            nc.sync.dma_start(qkvb_f[:], qkvb[:])
            qkvb_bf = const.tile([1, 3 * C], bf16, tag="qkvb_bf", name="qkvb_bf")
            nc.vector.tensor_copy(qkvb_bf[:], qkvb_f[:])
            pb_f = const.tile([1, C], fp32, tag="pb_f", name="pb_f")
            nc.sync.dma_start(pb_f[:], pb[:])
            pb_bf = const.tile([1, C], bf16, tag="pb_bf", name="pb_bf")
            nc.vector.tensor_copy(pb_bf[:], pb_f[:])
            qnwb = const.tile([D, 2], fp32, tag="qnwb", name="qnwb")
            nc.sync.dma_start(qnwb[:], qn_wb[:])
            knwb = const.tile([D, 2], fp32, tag="knwb", name="knwb")
            nc.sync.dma_start(knwb[:], kn_wb[:])

            # qT / kT-local / attnT accumulators (head-pair-major layout)
            qT_sb = [persist.tile([P, NLOC], bf16, tag=f"qT{p}", name=f"qT{p}") for p in range(HP)]
            attnT = [persist.tile([P, NLOC], bf16, tag=f"aT{p}", name=f"aT{p}") for p in range(HP)]

            kv_local = dram.tile([2 * NLOC, C], bf16, tag="kv_local", name="kv_local")
            kv_full = dram.tile([2 * N, C], bf16, tag="kv_full", name="kv_full")

            # ================= Phase A: QKV + LN + transposes =================
            with (
                tc.tile_pool(name="qkv_ps", bufs=3, space="PSUM") as qkv_ps,
                tc.tile_pool(name="tp_ps", bufs=2, space="PSUM") as tp_ps,
                tc.tile_pool(name="ln", bufs=2) as ln_pool,
                tc.tile_pool(name="kv_stage", bufs=2) as kv_stage,
                tc.tile_pool(name="pa_w", bufs=1) as pa_w,
            ):
                xT_sb = [pa_w.tile([P, NLOC], bf16, tag=f"xT{i}", name=f"xT{i}") for i in range(8)]
                for i in range(8):
                    nc.sync.dma_start(xT_sb[i][:], xT[i * P:(i + 1) * P, :])
                wq_sb = [pa_w.tile([P, 3 * C], bf16, tag=f"wq{i}", name=f"wq{i}") for i in range(8)]
                for i in range(8):
                    nc.sync.dma_start(wq_sb[i][:], wqkvT[i * P:(i + 1) * P, :])
                for i in range(NT):
                    q_f = ln_pool.tile([P, C], fp32, tag="q_f", name="q_f")
                    k_f = ln_pool.tile([P, C], fp32, tag="k_f", name="k_f")
                    v_bf = kv_stage.tile([P, C], bf16, tag="v_bf", name="v_bf")
                    for j in range(CH3):
                        ps = qkv_ps.tile([P, 512], fp32, tag="ps", name="ps")
                        nc.tensor.matmul(ps[:], ones_row[:, :P],
                                         qkvb_bf[:, j * 512:(j + 1) * 512],
                                         start=True, stop=False)
                        for kk in range(8):
                            nc.tensor.matmul(
                                ps[:],
                                xT_sb[kk][:, i * P:(i + 1) * P],
                                wq_sb[kk][:, j * 512:(j + 1) * 512],
                                start=False, stop=(kk == 7))
                        if j < 2:
                            nc.vector.tensor_copy(q_f[:, j * 512:(j + 1) * 512], ps[:])
                        elif j < 4:
                            nc.vector.tensor_copy(k_f[:, (j - 2) * 512:(j - 1) * 512], ps[:])
                        else:
                            nc.vector.tensor_copy(v_bf[:, (j - 4) * 512:(j - 3) * 512], ps[:])
                    # v straight out to kv_local rows [NLOC + i*128)
                    nc.sync.dma_start(kv_local[NLOC + i * P: NLOC + (i + 1) * P, :], v_bf[:])

                    for name, t_f, wb in (("q", q_f, qnwb), ("k", k_f, knwb)):
                        t3 = t_f[:].rearrange("p (h d) -> p h d", d=D)
                        sums = ln_pool.tile([P, H], fp32, tag=f"{name}sum", name=f"{name}sum")
                        nc.vector.tensor_reduce(sums[:], t3, axis=AX.X, op=ALU.add)
                        sq = ln_pool.tile([P, C], fp32, tag=f"{name}sq", name=f"{name}sq")
                        nc.scalar.activation(sq[:], t_f[:], AF.Square)
                        ssq = ln_pool.tile([P, H], fp32, tag=f"{name}ssq", name=f"{name}ssq")
                        nc.vector.tensor_reduce(
                            ssq[:], sq[:].rearrange("p (h d) -> p h d", d=D),
                            axis=AX.X, op=ALU.add)
                        mu = ln_pool.tile([P, H], fp32, tag=f"{name}mu", name=f"{name}mu")
                        nc.vector.tensor_scalar_mul(mu[:], sums[:], 1.0 / D)
                        mu2 = ln_pool.tile([P, H], fp32, tag=f"{name}mu2", name=f"{name}mu2")
                        nc.vector.tensor_mul(mu2[:], mu[:], mu[:])
                        var = ln_pool.tile([P, H], fp32, tag=f"{name}var", name=f"{name}var")
                        nc.vector.scalar_tensor_tensor(
                            var[:], ssq[:], 1.0 / D, mu2[:],
                            op0=ALU.mult, op1=ALU.subtract)
                        sig = ln_pool.tile([P, H], fp32, tag=f"{name}sig", name=f"{name}sig")
                        nc.scalar.activation(sig[:], var[:], AF.Sqrt, bias=eps_t[:])
                        rstd = ln_pool.tile([P, H], fp32, tag=f"{name}rstd", name=f"{name}rstd")
                        nc.vector.reciprocal(rstd[:], sig[:])
                        tn = ln_pool.tile([P, C], bf16, tag=f"{name}n", name=f"{name}n")
                        for h in range(H):
                            nc.vector.tensor_scalar(
                                tn[:, h * D:(h + 1) * D], t_f[:, h * D:(h + 1) * D],
                                mu[:, h:h + 1], rstd[:, h:h + 1],
                                op0=ALU.subtract, op1=ALU.mult)
                        # transpose per head into [d, n] layout (+ LN affine)
                        for h in range(H):
                            tp = tp_ps.tile([D, P], bf16, tag="tp", name="tp")
                            nc.tensor.transpose(tp[:], tn[:, h * D:(h + 1) * D], ident[:])
                            hp, hh = h // 2, h % 2
                            if name == "q":
                                nc.vector.tensor_scalar(
                                    qT_sb[hp][hh * D:(hh + 1) * D, i * P:(i + 1) * P],
                                    tp[:], wb[:, 0:1], wb[:, 1:2],
                                    op0=ALU.mult, op1=ALU.add)
                            else:
                                if h == 0:
                                    kT_stage = kv_stage.tile([P, C], bf16,
                                                             tag="kT_stage", name="kT_stage")
                                nc.vector.tensor_scalar(
                                    kT_stage[hh * D:(hh + 1) * D, hp * P:(hp + 1) * P],
                                    tp[:], wb[:, 0:1], wb[:, 1:2],
                                    op0=ALU.mult, op1=ALU.add)
                        if name == "k":
                            for hp in range(HP):
                                nc.sync.dma_start(
                                    kv_local[hp * P:(hp + 1) * P, i * P:(i + 1) * P],
                                    kT_stage[:, hp * P:(hp + 1) * P])

            # ================= Phase B: AllGather k^T | v =====================
            nc.gpsimd.collective_compute(
                "AllGather", mybir.AluOpType.bypass,
                replica_groups=rg,
                ins=[kv_local[:].opt()],
                outs=[kv_full[:].opt()],
            )

            # ================= Phase C: flash attention =======================
            SL = 2 * (D + 1)   # 130: [vA(64)|1|vB(64)|1] per key tile
            with (
                tc.tile_pool(name="st_ps", bufs=2, space="PSUM") as st_ps,
                tc.tile_pool(name="o_ps", bufs=2, space="PSUM") as o_ps,
                tc.tile_pool(name="kv_sb", bufs=2) as kv_sb,
                tc.tile_pool(name="p_sb", bufs=4) as p_sb,
                tc.tile_pool(name="nrm", bufs=2) as nrm,
            ):
                for hp in range(HP):
                    kThp = kv_sb.tile([P, N], bf16, tag="kThp", name="kThp")
                    for r in range(4):
                        nc.sync.dma_start(
                            kThp[:, r * NLOC:(r + 1) * NLOC],
                            kv_full[2 * NLOC * r + hp * P: 2 * NLOC * r + (hp + 1) * P, :])
                    vaug = kv_sb.tile([P, KT * SL], bf16, tag="vaug", name="vaug")
                    # ones-columns at 64, 129, 194, ... (step 65 covers both halves)
                    nc.vector.memset(vaug[:, D::(D + 1)], 1.0)
                    for r in range(4):
                        for hh in range(2):
                            nc.sync.dma_start(
                                vaug[:, r * 8 * SL:(r + 1) * 8 * SL].rearrange(
                                    "p (t d) -> p t d", d=SL)[
                                    :, :, hh * (D + 1): hh * (D + 1) + D],
                                kv_full[2 * NLOC * r + NLOC: 2 * NLOC * r + 2 * NLOC,
                                        hp * P + hh * D: hp * P + (hh + 1) * D].rearrange(
                                    "(t p) d -> p t d", p=P))
                    for m in range(2):
                        o_tiles = [o_ps.tile([D + 1, 512], fp32, tag=f"o{hh}", name=f"o{hh}")
                                   for hh in range(2)]
                        # software pipeline: S+exp run LEAD steps ahead of PV so
                        # PE never stalls in-order behind a pending exp.
                        LEAD = 2
                        pq = []  # pending (t2, hh, p_tile)

                        def issue_pv(t2p, hh, p_t):
                            for u in range(2):
                                t = 2 * t2p + u
                                nc.tensor.matmul(
                                    o_tiles[hh][:],
                                    vaug[:, t * SL + hh * (D + 1):
                                         t * SL + (hh + 1) * (D + 1)],
                                    p_t[:, u * 512:(u + 1) * 512],
                                    start=(t == 0), stop=(t == KT - 1))

                        for t2 in range(KT // 2):
                            for hh in range(2):
                                st = st_ps.tile([P, 1024], fp32, tag="st", name="st")
                                for u in range(2):
                                    t = 2 * t2 + u
                                    nc.tensor.matmul(
                                        st[:, u * 512:(u + 1) * 512],
                                        kThp[hh * D:(hh + 1) * D, t * P:(t + 1) * P],
                                        qT_sb[hp][hh * D:(hh + 1) * D,
                                                  m * 512:(m + 1) * 512],
                                        start=True, stop=True)
                                p_t = p_sb.tile([P, 1024], bf16, tag=f"p{hh}",
                                                name=f"p{hh}")
                                nc.scalar.activation(p_t[:], st[:], AF.Exp, scale=SCALE)
                                pq.append((t2, hh, p_t))
                            while len(pq) > 2 * LEAD:
                                issue_pv(*pq.pop(0))
                        for args in pq:
                            issue_pv(*args)
                        for hh in range(2):
                            linv = nrm.tile([1, 512], fp32, tag=f"li{hh}", name=f"li{hh}")
                            nc.vector.reciprocal(linv[:], o_tiles[hh][D:D + 1, :])
                            bc_sb = nrm.tile([D, 512], fp32, tag=f"bs{hh}", name=f"bs{hh}")
                            nc.gpsimd.partition_broadcast(bc_sb[:], linv[:], channels=D)
                            nc.vector.tensor_mul(
                                attnT[hp][hh * D:(hh + 1) * D, m * 512:(m + 1) * 512],
                                o_tiles[hh][0:D, :], bc_sb[:])

            # ================= Phase D: output projection =====================
            with (
                tc.tile_pool(name="y_ps", bufs=2, space="PSUM") as y_ps,
                tc.tile_pool(name="y_sb", bufs=2) as y_sb_pool,
                tc.tile_pool(name="pd_w", bufs=1) as pd_w,
            ):
                wp_sb = [pd_w.tile([P, C], bf16, tag=f"wp{i}", name=f"wp{i}") for i in range(8)]
                for i in range(8):
                    nc.sync.dma_start(wp_sb[i][:], wpT[i * P:(i + 1) * P, :])
                for i in range(NT):
                    y_sb = y_sb_pool.tile([P, C], fp32, tag="y", name="y")
                    for co in range(2):
                        yp = y_ps.tile([P, 512], fp32, tag="yp", name="yp")
                        nc.tensor.matmul(yp[:], ones_row[:, :P],
                                         pb_bf[:, co * 512:(co + 1) * 512],
                                         start=True, stop=False)
                        for p in range(8):
                            nc.tensor.matmul(
                                yp[:],
                                attnT[p][:, i * P:(i + 1) * P],
                                wp_sb[p][:, co * 512:(co + 1) * 512],
                                start=False, stop=(p == 7))
                        nc.vector.tensor_copy(y_sb[:, co * 512:(co + 1) * 512], yp[:])
                    nc.sync.dma_start(out[i * P:(i + 1) * P, :], y_sb[:])

    nc.finalize()
    return nc


def _prep_in_maps(x, qkv_w, qkv_b, q_norm_w, q_norm_b, k_norm_w, k_norm_b,
                  proj_w, proj_b):
    wqkvT = np.ascontiguousarray(qkv_w.T).astype(BF16)
    wpT = np.ascontiguousarray(proj_w.T).astype(BF16)
    qkvb = qkv_b.reshape(1, 3 * C).astype(np.float32)
    pb = proj_b.reshape(1, C).astype(np.float32)
    qn_wb = np.stack([q_norm_w, q_norm_b], axis=1).astype(np.float32)
    kn_wb = np.stack([k_norm_w, k_norm_b], axis=1).astype(np.float32)
    in_maps = []
    for c in range(N_CORES):
        b, s = c // 4, c % 4
        xt = np.ascontiguousarray(x[b, s * NLOC:(s + 1) * NLOC, :].T).astype(BF16)
        in_maps.append({
            "xT": xt, "wqkvT": wqkvT, "qkvb": qkvb, "wpT": wpT, "pb": pb,
            "qn_wb": qn_wb, "kn_wb": kn_wb,
        })
    return in_maps


def _install_ntff_hook_shim():
    """The agent image's antenv lacks axon_hooks; recreate it so trace=True
    can register the NTFF profile hook that trn_boot would have set."""
    import types
    import antenv

    if "antenv.axon_hooks" in sys.modules:
        return
    mod = types.ModuleType("antenv.axon_hooks")
    state = {"fn": None}
    mod.set_axon_ntff_profile_hook = lambda fn: state.__setitem__("fn", fn)
    mod.get_axon_ntff_profile_hook = lambda: state["fn"]
    sys.modules["antenv.axon_hooks"] = mod
    antenv.axon_hooks = mod
    try:
        from trn_agent_boot.trn_boot import _ntff_profile_via_ctypes
        hook = _ntff_profile_via_ctypes("/opt/axon/libaxon_pjrt.so")
        if hook is not None:
            mod.set_axon_ntff_profile_hook(hook)
    except Exception as e:  # degrade to no tracing
        print(f"ntff hook shim failed: {e}", file=sys.stderr)


def kernel(x, qkv_w, qkv_b, q_norm_w, q_norm_b, k_norm_w, k_norm_b,
           proj_w, proj_b, _trace=False):
    from concourse.bass_utils import run_bass_kernel_spmd

    if _trace:
        _install_ntff_hook_shim()

    if "nc" not in _COMPILED:
        _COMPILED["nc"] = build_graph()
    nc = _COMPILED["nc"]

    in_maps = _prep_in_maps(x, qkv_w, qkv_b, q_norm_w, q_norm_b,
                            k_norm_w, k_norm_b, proj_w, proj_b)
    res = run_bass_kernel_spmd(nc, in_maps, core_ids=list(range(N_CORES)),
                               trace=_trace)
    out = np.empty((B, N, C), dtype=np.float32)
    for c in range(N_CORES):
        b, s = c // 4, c % 4
        out[b, s * NLOC:(s + 1) * NLOC, :] = res.results[c]["out"]
    if _trace:
        _COMPILED["last_exec_time_ns"] = res.exec_time_ns
        _COMPILED["last_results"] = res
    return out
